# revision 19
# baseline (speedup 1.0000x reference)
"""Trainium2 Bass kernel for a 2-layer dense GCN block:

    z = x.reshape(B, N, F)                     # B=4, N=8192, F=64
    for i in range(2):
        z = relu((A @ z) @ W_i)                # A: [N, N] dense
    return z

Strategy (8 NeuronCores, SPMD):
  * Shard the output rows (m) of A @ Z across cores: core j owns rows
    [1024*j, 1024*(j+1)).  The host hands core j the matching
    column-slice of A^T (contraction dim n on SBUF partitions), cast to
    bf16 and pre-swizzled into exact SBUF tile order so every chunk DMA
    is one flat contiguous copy.  The 16 MB shard stays resident in
    SBUF for BOTH layers -- A is read from HBM exactly once.
  * Z is a [n, c] matrix with c = b*F + f (256 columns).  Layer matmuls
    compute H^T[c, m] = sum_n Z[n, c] * A^T[n, m] on the tensor engine
    (lhsT = Z tile stationary, rhs = A^T tile moving, fp32 PSUM accum).
  * The inter-layer exchange (ncfw AllGather of each core's z1 slice)
    is the critical path: collectives complete serially ~20 us apart
    and the first lands ~95-120 us into the kernel no matter how early
    it is triggered.  So the kernel exposes ONLY the first gather:
      - layer 1 runs as four m-quarter passes; the m-half gather g01
        (256 KB in) triggers ~1/2 through layer 1, then g2 / g3.
      - layer 2 runs as paired sub-passes over gather phases: both
        m-half accumulations sweep g01's 32 n-tiles first (33.6 us of
        PE work needing only the first gather), then g2's 16, then
        g3's 16 -- late gathers get enormous deadline slack.
  * Ring discipline: z0 + z1_loc stores + outputs on the sync HWDGE
    ring (a gather's input store gates its trigger doorbell and must
    never queue behind the A load); A + gather reloads on the scalar
    ring, with reloads at LOW scheduler priority so one parked on a
    collective wait can never sit ahead of A chunks in the ring FIFO.
  * Layer-2 accumulation matmuls are emitted at LOW priority so
    layer-1 tail work always precedes them in the PE queue.
  * Both c-halves of an accumulation share one PSUM bank (per-element
    has_written semantics, single start=True per bank); layers use
    disjoint tags so a pass tail can't be ordered behind the next
    layer's matmuls.  PSUM: 2 (L1) + 4 (L2) + 2 (apply) = 8 banks.
  * bf16 operands / fp32 accumulation (~0.5% rel-l2 vs fp32 ref).
"""

import contextlib

import numpy as np
import ml_dtypes

import concourse.mybir as mybir
import concourse.tile as tile
from concourse import bacc
from concourse.bass_utils import run_bass_kernel_spmd

BF16 = ml_dtypes.bfloat16

NCORES = 8
B, N, F, L = 4, 8192, 64, 2
C = B * F                      # 256 columns of the Z matrix
M_CORE = N // NCORES           # 1024 output rows per core
NT = N // 128                  # 64 contraction tiles of 128
MT = M_CORE // 128             # 8 output-row tiles of 128 per core
KQ = [4, 4, 4, 4]              # A chunks per quarter (2 MB each)
ZCH = 8                        # DMA chunks for z0
TPZ = NT // ZCH                # 4 n-tiles per z chunk
NQ = 4                         # m-quarter passes of layer 1
MPG = MT // NQ                 # m-tiles per quarter (2)
MQ = M_CORE // NQ              # m columns per quarter (256)
# gather phases: slice 0 = m-half 0 (quarters 0,1), slices 1,2 = q2,q3
GSLICE = [(0, 4), (4, 6), (6, 8)]   # m-tile ranges of the 3 AllGathers

_CACHED = {}


def _build_program():
    nc = bacc.Bacc("TRN2", target_bir_lowering=False, debug=False,
                   num_devices=NCORES)
    dt = mybir.dt

    at_d = nc.dram_tensor("at", [sum(KQ) * 128, max(NT // k for k in KQ) * MQ],
                          dt.bfloat16, kind="ExternalInput")
    z0_d = nc.dram_tensor("z0", [ZCH * 128, TPZ * C], dt.bfloat16,
                          kind="ExternalInput")
    w_d = nc.dram_tensor("w", [128, 2 * 128], dt.bfloat16, kind="ExternalInput")
    out_d = nc.dram_tensor("out", [M_CORE, C], dt.bfloat16, kind="ExternalOutput")

    z1_loc = nc.dram_tensor("z1_loc", [M_CORE, C], dt.bfloat16)
    warm_in = nc.dram_tensor("warm_in", [1, 128], dt.bfloat16)
    warm_out = nc.dram_tensor("warm_out", [NCORES, 128], dt.bfloat16,
                              addr_space="Shared")
    z1g = [nc.dram_tensor(f"z1g{g}", [NCORES * (hi - lo) * 128, C],
                          dt.bfloat16, addr_space="Shared")
           for g, (lo, hi) in enumerate(GSLICE)]

    with tile.TileContext(nc) as tc:
        with tc.tile_pool(name="a_res", bufs=1) as a_pool, \
             tc.tile_pool(name="z_res", bufs=1) as z_pool, \
             tc.tile_pool(name="z1_res", bufs=1) as z1_pool, \
             tc.tile_pool(name="wk", bufs=1) as w_pool, \
             tc.tile_pool(name="ps", bufs=1, space="PSUM") as ps_pool, \
             tc.tile_pool(name="pz", bufs=2, space="PSUM") as psz_pool, \
             tc.tile_pool(name="hsb", bufs=2) as hsb_pool, \
             tc.tile_pool(name="zout", bufs=4) as zout_pool:

            # Warm the ncfw collective path FIRST -- emitted before any
            # load DMA so its rounds run before HBM saturates.  The
            # first collective pays a large one-time cost; this tiny
            # warmup absorbs it under layer 1.
            nc.gpsimd.dma_start(out=warm_in[:], in_=z0_d[0:1, 0:128])
            nc.gpsimd.collective_compute(
                "AllGather",
                mybir.AluOpType.bypass,
                replica_groups=[list(range(NCORES))],
                ins=[warm_in.ap().opt()],
                outs=[warm_out.ap().opt()],
            )

            w_sb = w_pool.tile([128, 2 * 128], dt.bfloat16, tag="w")
            nc.scalar.dma_start(out=w_sb[:], in_=w_d[:])

            at_sb = [[a_pool.tile([128, (NT // KQ[q]) * MQ], dt.bfloat16,
                                  tag=f"at{q}_{k}", name=f"at_sb{q}_{k}")
                      for k in range(KQ[q])] for q in range(NQ)]
            z_sb = [z_pool.tile([128, TPZ * C], dt.bfloat16,
                                tag=f"z{k}", name=f"z_sb{k}")
                    for k in range(ZCH)]
            z1_sb = [z1_pool.tile([128, NCORES * (hi - lo) * C], dt.bfloat16,
                                  tag=f"z1g{g}", name=f"z1_sb{g}")
                     for g, (lo, hi) in enumerate(GSLICE)]

            for k in range(ZCH):
                nc.sync.dma_start(out=z_sb[k][:],
                                  in_=z0_d[k * 128:(k + 1) * 128, :])
            # A load split across BOTH HWDGE rings (aggregate ~HBM rate):
            # quarters 0/1 (needed first) on the scalar ring, 2/3 behind
            # z0 on the sync ring.
            row = 0
            for q in range(NQ):
                cols = (NT // KQ[q]) * MQ
                eng = nc.scalar if q < 2 else nc.sync
                for k in range(KQ[q]):
                    eng.dma_start(out=at_sb[q][k][:],
                                  in_=at_d[row:row + 128, 0:cols])
                    row += 128

            def z_tile(t, ch):
                """lhsT: Z[n-tile t, c-half ch] -> [128, 128] bf16."""
                k, tt = divmod(t, TPZ)
                return z_sb[k][:, tt * C + ch * 128: tt * C + ch * 128 + 128]

            def z2_tile(t, ch):
                """Same, from the gathered z1 slices."""
                cb, r = divmod(t, MT)
                g = next(i for i, (lo, hi) in enumerate(GSLICE) if lo <= r < hi)
                lo, hi = GSLICE[g]
                blk = cb * (hi - lo) + (r - lo)
                return z1_sb[g][:, blk * C + ch * 128: blk * C + ch * 128 + 128]

            def at_tile(t, q):
                """rhs: A^T[n-tile t, m-quarter q] -> [128, 256] bf16."""
                k, tt = divmod(t, NT // KQ[q])
                return at_sb[q][k][:, tt * MQ:(tt + 1) * MQ]

            h_sb = [hsb_pool.tile([128, M_CORE], dt.bfloat16,
                                  tag=f"h{ch}", name=f"h_sb{ch}")
                    for ch in range(2)]

            def tail(li, qs, h_ap, on_slice_done, prio):
                # weight apply + relu + store for the m-tiles of a
                # just-finished pass; overlaps the next pass's matmuls.
                with prio:
                    for ch in range(2):
                        for q in qs:
                            nc.vector.tensor_copy(
                                h_sb[ch][:, q * MQ:(q + 1) * MQ],
                                h_ap(ch, q),
                            )
                    for g in qs:
                        z_ps = psz_pool.tile([128, MPG * C], dt.float32,
                                             tag="zps", name=f"z_ps_{li}_{g}")
                        for j in range(MPG):
                            i = g * MPG + j
                            for ch in range(2):
                                nc.tensor.matmul(
                                    z_ps[:, j * C + ch * 128:
                                         j * C + (ch + 1) * 128],
                                    h_sb[ch][:, i * 128:(i + 1) * 128],
                                    w_sb[:, li * 128:(li + 1) * 128],
                                    start=(j == 0 and ch == 0), stop=True,
                                )
                        z_o = zout_pool.tile([128, MPG * C], dt.bfloat16,
                                             tag="zo", name=f"z_o_{li}_{g}")
                        nc.scalar.activation(z_o[:], z_ps[:],
                                             mybir.ActivationFunctionType.Relu)
                        on_slice_done(g, z_o)

            # ---- layer 1: four m-quarter passes ----
            def l1_slice_done(q, z_o):
                nc.scalar.dma_start(
                    out=z1_loc.ap()[q * MPG * 128:(q + 1) * MPG * 128, :]
                        .rearrange("(t p) c -> p t c", p=128),
                    in_=z_o.rearrange("p (t c) -> p t c", c=C))
                gs = {1: 0, 2: 1, 3: 2}  # quarter -> gather launched after it
                if q in gs:
                    g = gs[q]
                    lo, hi = GSLICE[g]
                    nc.gpsimd.collective_compute(
                        "AllGather",
                        mybir.AluOpType.bypass,
                        replica_groups=[list(range(NCORES))],
                        ins=[z1_loc.ap()[lo * 128:hi * 128, :].opt()],
                        outs=[z1g[g].ap().opt()],
                    )
                    # Low priority: a reload parked on its collective
                    # wait must never sit ahead of A chunks in the
                    # scalar ring FIFO.
                    with tc.high_priority(offset=-1_000_000):
                        nc.gpsimd.dma_start(
                            out=z1_sb[g].rearrange(
                                "p (cb t c) -> p cb t c",
                                cb=NCORES, t=hi - lo),
                            in_=z1g[g].ap().rearrange(
                                "(cb t p) c -> p cb t c",
                                cb=NCORES, p=128))

            l1_ps = [ps_pool.tile([128, 2 * MQ], dt.float32, tag=f"hl1_{par}",
                                  name=f"l1_ps{par}") for par in range(2)]
            for q in range(NQ):
                par = q % 2
                for ti, t in enumerate(range(NT)):
                    for ch in range(2):
                        nc.tensor.matmul(
                            l1_ps[par][:, ch * MQ:(ch + 1) * MQ],
                            z_tile(t, ch),
                            at_tile(t, q),
                            start=(ti == 0 and ch == 0),
                            stop=(ti == NT - 1),
                        )
                tail(0, [q],
                     lambda ch, _q, par=par: l1_ps[par][:, ch * MQ:(ch + 1) * MQ],
                     l1_slice_done, tc.high_priority())

            # ---- layer 2: paired m-half sub-passes per gather phase ----
            tg = [[MT * cb + r for cb in range(NCORES) for r in range(lo, hi)]
                  for (lo, hi) in GSLICE]

            def l2_slice_done(g, z_o):
                nc.sync.dma_start(
                    out=out_d.ap()[g * MPG * 128:(g + 1) * MPG * 128, :]
                        .rearrange("(t p) c -> p t c", p=128),
                    in_=z_o.rearrange("p (t c) -> p t c", c=C))

            l2_ps = [[ps_pool.tile([128, 2 * MQ], dt.float32,
                                   tag=f"hl2_{mh}{ch}", name=f"l2_ps_{mh}{ch}")
                      for ch in range(2)] for mh in range(2)]

            def l2_subpass(gp, mh):
                qs = (2 * mh, 2 * mh + 1)
                with tc.high_priority(offset=-1_000_000):
                    for ti, t in enumerate(tg[gp]):
                        for ch in range(2):
                            for q in qs:
                                nc.tensor.matmul(
                                    l2_ps[mh][ch][:, (q % 2) * MQ:
                                                  (q % 2 + 1) * MQ],
                                    z2_tile(t, ch),
                                    at_tile(t, q),
                                    start=(gp == 0 and ti == 0 and q == qs[0]),
                                    stop=(gp == 2 and ti == len(tg[2]) - 1),
                                )

            for gp in range(2):
                for mh in range(2):
                    l2_subpass(gp, mh)
            l2_subpass(2, 0)
            tail(1, [0, 1],
                 lambda ch, q: l2_ps[0][ch][:, (q % 2) * MQ:(q % 2 + 1) * MQ],
                 l2_slice_done, contextlib.nullcontext())
            l2_subpass(2, 1)
            tail(1, [2, 3],
                 lambda ch, q: l2_ps[1][ch][:, (q % 2) * MQ:(q % 2 + 1) * MQ],
                 l2_slice_done, contextlib.nullcontext())

    nc.compile()
    return nc


def _prep_inputs(x, net_params, A):
    a_bf = A.astype(BF16)
    z0 = np.ascontiguousarray(x.transpose(1, 0, 2).reshape(N, C)).astype(BF16)
    # z0 in SBUF tile order [k, p, t, c] -> [ZCH*128, TPZ*C]
    z0_sw = np.ascontiguousarray(
        z0.reshape(ZCH, TPZ, 128, C).transpose(0, 2, 1, 3)
    ).reshape(ZCH * 128, TPZ * C)
    w = net_params.astype(np.float32).reshape(L, F, F).astype(BF16)
    # block-diagonal weight tile per layer: diag(W_l, W_l)
    w_sb = np.zeros((128, 2 * 128), dtype=BF16)
    for li in range(L):
        w_sb[0:F, li * 128:li * 128 + F] = w[li]
        w_sb[F:2 * F, li * 128 + F:li * 128 + 2 * F] = w[li]
    max_cols = max(NT // k for k in KQ) * MQ
    in_maps = []
    for j in range(NCORES):
        at_j = np.ascontiguousarray(a_bf[j * M_CORE:(j + 1) * M_CORE, :].T)
        # per-quarter chunking in SBUF tile order [q][k][p, t, m], padded
        # to the widest chunk in the free dim
        at_sw = np.zeros((sum(KQ) * 128, max_cols), dtype=BF16)
        row = 0
        for q in range(NQ):
            tpc = NT // KQ[q]
            blk = at_j[:, q * MQ:(q + 1) * MQ].reshape(KQ[q], tpc, 128, MQ)
            blk = blk.transpose(0, 2, 1, 3).reshape(KQ[q] * 128, tpc * MQ)
            at_sw[row:row + KQ[q] * 128, 0:tpc * MQ] = blk
            row += KQ[q] * 128
        in_maps.append({"at": at_sw, "z0": z0_sw, "w": w_sb})
    return in_maps


def kernel(x, t, net_params, A):
    x = np.asarray(x)
    A = np.asarray(A)
    net_params = np.asarray(net_params)

    if "nc" not in _CACHED:
        _CACHED["nc"] = _build_program()
    nc = _CACHED["nc"]

    in_maps = _prep_inputs(x, net_params, A)
    _CACHED["in_maps"] = in_maps
    res = run_bass_kernel_spmd(nc, in_maps, list(range(NCORES)))
    full = np.concatenate([res.results[c]["out"] for c in range(NCORES)],
                          axis=0).astype(np.float32)
    return np.ascontiguousarray(full.reshape(N, B, F).transpose(1, 0, 2))


# revision 20
# speedup vs baseline: 1.2708x; 1.2708x over previous
"""Trainium2 Bass kernel for a 2-layer dense GCN block:

    z = x.reshape(B, N, F)                     # B=4, N=8192, F=64
    for i in range(2):
        z = relu((A @ z) @ W_i)                # A: [N, N] dense
    return z

Strategy (8 NeuronCores, SPMD):
  * Shard the output rows (m) of A @ Z across cores: core j owns rows
    [1024*j, 1024*(j+1)).  The host hands core j the matching
    column-slice of A^T (contraction dim n on SBUF partitions), cast to
    bf16 and pre-swizzled into exact SBUF tile order so every chunk DMA
    is one flat contiguous copy.  The 16 MB shard stays resident in
    SBUF for BOTH layers -- A is read from HBM exactly once.
  * Z is a [n, c] matrix with c = b*F + f (256 columns).  Layer matmuls
    compute H^T[c, m] = sum_n Z[n, c] * A^T[n, m] on the tensor engine
    (lhsT = Z tile stationary, rhs = A^T tile moving, fp32 PSUM accum).
  * The inter-layer exchange (ncfw AllGather of each core's z1 slice)
    is the critical path: collectives complete serially ~20 us apart
    and the first lands ~95-120 us into the kernel no matter how early
    it is triggered.  So the kernel exposes ONLY the first gather:
      - layer 1 runs as four m-quarter passes; the m-half gather g01
        (256 KB in) triggers ~1/2 through layer 1, then g2 / g3.
      - layer 2 runs as paired sub-passes over gather phases: both
        m-half accumulations sweep g01's 32 n-tiles first (33.6 us of
        PE work needing only the first gather), then g2's 16, then
        g3's 16 -- late gathers get enormous deadline slack.
  * Ring discipline: z0 + z1_loc stores + outputs on the sync HWDGE
    ring (a gather's input store gates its trigger doorbell and must
    never queue behind the A load); A + gather reloads on the scalar
    ring, with reloads at LOW scheduler priority so one parked on a
    collective wait can never sit ahead of A chunks in the ring FIFO.
  * Layer-2 accumulation matmuls are emitted at LOW priority so
    layer-1 tail work always precedes them in the PE queue.
  * Both c-halves of an accumulation share one PSUM bank (per-element
    has_written semantics, single start=True per bank); layers use
    disjoint tags so a pass tail can't be ordered behind the next
    layer's matmuls.  PSUM: 2 (L1) + 4 (L2) + 2 (apply) = 8 banks.
  * bf16 operands / fp32 accumulation (~0.5% rel-l2 vs fp32 ref).
"""

import contextlib

import numpy as np
import ml_dtypes

import concourse.mybir as mybir
import concourse.tile as tile
from concourse import bacc
from concourse.bass_utils import run_bass_kernel_spmd

BF16 = ml_dtypes.bfloat16

NCORES = 8
B, N, F, L = 4, 8192, 64, 2
C = B * F                      # 256 columns of the Z matrix
M_CORE = N // NCORES           # 1024 output rows per core
NT = N // 128                  # 64 contraction tiles of 128
MT = M_CORE // 128             # 8 output-row tiles of 128 per core
KQ = [4, 4, 4, 4]              # A chunks per quarter (2 MB each)
ZCH = 8                        # DMA chunks for z0
TPZ = NT // ZCH                # 4 n-tiles per z chunk
NQ = 4                         # m-quarter passes of layer 1
MPG = MT // NQ                 # m-tiles per quarter (2)
MQ = M_CORE // NQ              # m columns per quarter (256)
# gather phases: two m-half AllGathers (after quarter 1 and quarter 3)
GSLICE = [(0, 4), (4, 8)]           # m-tile ranges of the AllGathers

_CACHED = {}


def _build_program():
    nc = bacc.Bacc("TRN2", target_bir_lowering=False, debug=False,
                   num_devices=NCORES)
    dt = mybir.dt

    at_d = nc.dram_tensor("at", [sum(KQ) * 128, max(NT // k for k in KQ) * MQ],
                          dt.bfloat16, kind="ExternalInput")
    z0_d = nc.dram_tensor("z0", [ZCH * 128, TPZ * C], dt.bfloat16,
                          kind="ExternalInput")
    w_d = nc.dram_tensor("w", [128, 2 * 128], dt.bfloat16, kind="ExternalInput")
    out_d = nc.dram_tensor("out", [M_CORE, C], dt.bfloat16, kind="ExternalOutput")

    z1_loc = nc.dram_tensor("z1_loc", [M_CORE, C], dt.bfloat16)
    warm_in = nc.dram_tensor("warm_in", [1, 128], dt.bfloat16)
    warm_out = nc.dram_tensor("warm_out", [NCORES, 128], dt.bfloat16,
                              addr_space="Shared")
    z1g = [nc.dram_tensor(f"z1g{g}", [NCORES * (hi - lo) * 128, C],
                          dt.bfloat16, addr_space="Shared")
           for g, (lo, hi) in enumerate(GSLICE)]

    with tile.TileContext(nc) as tc:
        with tc.tile_pool(name="a_res", bufs=1) as a_pool, \
             tc.tile_pool(name="z_res", bufs=1) as z_pool, \
             tc.tile_pool(name="z1_res", bufs=1) as z1_pool, \
             tc.tile_pool(name="wk", bufs=1) as w_pool, \
             tc.tile_pool(name="ps", bufs=1, space="PSUM") as ps_pool, \
             tc.tile_pool(name="pz", bufs=2, space="PSUM") as psz_pool, \
             tc.tile_pool(name="hsb", bufs=2) as hsb_pool, \
             tc.tile_pool(name="zout", bufs=4) as zout_pool:

            # Warm the ncfw collective path FIRST -- emitted before any
            # load DMA so its rounds run before HBM saturates.  The
            # first collective pays a large one-time cost; this tiny
            # warmup absorbs it under layer 1.
            nc.gpsimd.dma_start(out=warm_in[:], in_=z0_d[0:1, 0:128])
            nc.gpsimd.collective_compute(
                "AllGather",
                mybir.AluOpType.bypass,
                replica_groups=[list(range(NCORES))],
                ins=[warm_in.ap().opt()],
                outs=[warm_out.ap().opt()],
            )

            w_sb = w_pool.tile([128, 2 * 128], dt.bfloat16, tag="w")
            nc.scalar.dma_start(out=w_sb[:], in_=w_d[:])

            at_sb = [[a_pool.tile([128, (NT // KQ[q]) * MQ], dt.bfloat16,
                                  tag=f"at{q}_{k}", name=f"at_sb{q}_{k}")
                      for k in range(KQ[q])] for q in range(NQ)]
            z_sb = [z_pool.tile([128, TPZ * C], dt.bfloat16,
                                tag=f"z{k}", name=f"z_sb{k}")
                    for k in range(ZCH)]
            z1_sb = [z1_pool.tile([128, NCORES * (hi - lo) * C], dt.bfloat16,
                                  tag=f"z1g{g}", name=f"z1_sb{g}")
                     for g, (lo, hi) in enumerate(GSLICE)]

            for k in range(ZCH):
                nc.sync.dma_start(out=z_sb[k][:],
                                  in_=z0_d[k * 128:(k + 1) * 128, :])
            # Single-ring load (sync), in consumption order: leaves the
            # other ring + SDMA slots free, which correlates with much
            # faster ncfw collective progress under load.
            row = 0
            for q in range(NQ):
                cols = (NT // KQ[q]) * MQ
                for k in range(KQ[q]):
                    nc.sync.dma_start(out=at_sb[q][k][:],
                                      in_=at_d[row:row + 128, 0:cols])
                    row += 128

            def z_tile(t, ch):
                """lhsT: Z[n-tile t, c-half ch] -> [128, 128] bf16."""
                k, tt = divmod(t, TPZ)
                return z_sb[k][:, tt * C + ch * 128: tt * C + ch * 128 + 128]

            def z2_tile(t, ch):
                """Same, from the gathered z1 slices."""
                cb, r = divmod(t, MT)
                g = next(i for i, (lo, hi) in enumerate(GSLICE) if lo <= r < hi)
                lo, hi = GSLICE[g]
                blk = cb * (hi - lo) + (r - lo)
                return z1_sb[g][:, blk * C + ch * 128: blk * C + ch * 128 + 128]

            def at_tile(t, q):
                """rhs: A^T[n-tile t, m-quarter q] -> [128, 256] bf16."""
                k, tt = divmod(t, NT // KQ[q])
                return at_sb[q][k][:, tt * MQ:(tt + 1) * MQ]

            h_sb = [hsb_pool.tile([128, M_CORE], dt.bfloat16,
                                  tag=f"h{ch}", name=f"h_sb{ch}")
                    for ch in range(2)]

            def tail(li, qs, h_ap, on_slice_done, prio):
                # weight apply + relu + store for the m-tiles of a
                # just-finished pass; overlaps the next pass's matmuls.
                with prio:
                    for ch in range(2):
                        for q in qs:
                            nc.vector.tensor_copy(
                                h_sb[ch][:, q * MQ:(q + 1) * MQ],
                                h_ap(ch, q),
                            )
                    for g in qs:
                        z_ps = psz_pool.tile([128, MPG * C], dt.float32,
                                             tag="zps", name=f"z_ps_{li}_{g}")
                        for j in range(MPG):
                            i = g * MPG + j
                            for ch in range(2):
                                nc.tensor.matmul(
                                    z_ps[:, j * C + ch * 128:
                                         j * C + (ch + 1) * 128],
                                    h_sb[ch][:, i * 128:(i + 1) * 128],
                                    w_sb[:, li * 128:(li + 1) * 128],
                                    start=(j == 0 and ch == 0), stop=True,
                                )
                        z_o = zout_pool.tile([128, MPG * C], dt.bfloat16,
                                             tag="zo", name=f"z_o_{li}_{g}")
                        nc.scalar.activation(z_o[:], z_ps[:],
                                             mybir.ActivationFunctionType.Relu)
                        on_slice_done(g, z_o)

            # ---- layer 1: four m-quarter passes ----
            def l1_slice_done(q, z_o):
                nc.scalar.dma_start(
                    out=z1_loc.ap()[q * MPG * 128:(q + 1) * MPG * 128, :]
                        .rearrange("(t p) c -> p t c", p=128),
                    in_=z_o.rearrange("p (t c) -> p t c", c=C))
                gs = {1: 0, 3: 1}  # quarter -> gather launched after it
                if q in gs:
                    g = gs[q]
                    lo, hi = GSLICE[g]
                    nc.gpsimd.collective_compute(
                        "AllGather",
                        mybir.AluOpType.bypass,
                        replica_groups=[list(range(NCORES))],
                        ins=[z1_loc.ap()[lo * 128:hi * 128, :].opt()],
                        outs=[z1g[g].ap().opt()],
                    )
                    # Reload in emission order on the gpsimd stream:
                    # its wait on the collective also gates the NEXT
                    # gather's trigger (ncfw wedges beyond 2 pending).
                    nc.gpsimd.dma_start(
                        out=z1_sb[g].rearrange(
                            "p (cb t c) -> p cb t c",
                            cb=NCORES, t=hi - lo),
                        in_=z1g[g].ap().rearrange(
                            "(cb t p) c -> p cb t c",
                            cb=NCORES, p=128))

            l1_ps = [ps_pool.tile([128, 2 * MQ], dt.float32, tag=f"hl1_{par}",
                                  name=f"l1_ps{par}") for par in range(2)]
            for q in range(NQ):
                par = q % 2
                for ti, t in enumerate(range(NT)):
                    for ch in range(2):
                        nc.tensor.matmul(
                            l1_ps[par][:, ch * MQ:(ch + 1) * MQ],
                            z_tile(t, ch),
                            at_tile(t, q),
                            start=(ti == 0 and ch == 0),
                            stop=(ti == NT - 1),
                        )
                tail(0, [q],
                     lambda ch, _q, par=par: l1_ps[par][:, ch * MQ:(ch + 1) * MQ],
                     l1_slice_done, tc.high_priority())

            # ---- layer 2: paired m-half sub-passes per gather phase ----
            tg = [[MT * cb + r for cb in range(NCORES) for r in range(lo, hi)]
                  for (lo, hi) in GSLICE]

            def l2_slice_done(g, z_o):
                nc.sync.dma_start(
                    out=out_d.ap()[g * MPG * 128:(g + 1) * MPG * 128, :]
                        .rearrange("(t p) c -> p t c", p=128),
                    in_=z_o.rearrange("p (t c) -> p t c", c=C))

            l2_ps = [[ps_pool.tile([128, 2 * MQ], dt.float32,
                                   tag=f"hl2_{mh}{ch}", name=f"l2_ps_{mh}{ch}")
                      for ch in range(2)] for mh in range(2)]

            def l2_subpass(gp, mh):
                qs = (2 * mh, 2 * mh + 1)
                with tc.high_priority(offset=-1_000_000):
                    for ti, t in enumerate(tg[gp]):
                        for ch in range(2):
                            for q in qs:
                                nc.tensor.matmul(
                                    l2_ps[mh][ch][:, (q % 2) * MQ:
                                                  (q % 2 + 1) * MQ],
                                    z2_tile(t, ch),
                                    at_tile(t, q),
                                    start=(gp == 0 and ti == 0 and q == qs[0]),
                                    stop=(gp == 1 and ti == len(tg[1]) - 1),
                                )

            l2_subpass(0, 0)
            l2_subpass(0, 1)
            l2_subpass(1, 0)
            tail(1, [0, 1],
                 lambda ch, q: l2_ps[0][ch][:, (q % 2) * MQ:(q % 2 + 1) * MQ],
                 l2_slice_done, contextlib.nullcontext())
            l2_subpass(1, 1)
            tail(1, [2, 3],
                 lambda ch, q: l2_ps[1][ch][:, (q % 2) * MQ:(q % 2 + 1) * MQ],
                 l2_slice_done, contextlib.nullcontext())

    nc.compile()
    return nc


def _prep_inputs(x, net_params, A):
    a_bf = A.astype(BF16)
    z0 = np.ascontiguousarray(x.transpose(1, 0, 2).reshape(N, C)).astype(BF16)
    # z0 in SBUF tile order [k, p, t, c] -> [ZCH*128, TPZ*C]
    z0_sw = np.ascontiguousarray(
        z0.reshape(ZCH, TPZ, 128, C).transpose(0, 2, 1, 3)
    ).reshape(ZCH * 128, TPZ * C)
    w = net_params.astype(np.float32).reshape(L, F, F).astype(BF16)
    # block-diagonal weight tile per layer: diag(W_l, W_l)
    w_sb = np.zeros((128, 2 * 128), dtype=BF16)
    for li in range(L):
        w_sb[0:F, li * 128:li * 128 + F] = w[li]
        w_sb[F:2 * F, li * 128 + F:li * 128 + 2 * F] = w[li]
    max_cols = max(NT // k for k in KQ) * MQ
    in_maps = []
    for j in range(NCORES):
        at_j = np.ascontiguousarray(a_bf[j * M_CORE:(j + 1) * M_CORE, :].T)
        # per-quarter chunking in SBUF tile order [q][k][p, t, m], padded
        # to the widest chunk in the free dim
        at_sw = np.zeros((sum(KQ) * 128, max_cols), dtype=BF16)
        row = 0
        for q in range(NQ):
            tpc = NT // KQ[q]
            blk = at_j[:, q * MQ:(q + 1) * MQ].reshape(KQ[q], tpc, 128, MQ)
            blk = blk.transpose(0, 2, 1, 3).reshape(KQ[q] * 128, tpc * MQ)
            at_sw[row:row + KQ[q] * 128, 0:tpc * MQ] = blk
            row += KQ[q] * 128
        in_maps.append({"at": at_sw, "z0": z0_sw, "w": w_sb})
    return in_maps


def kernel(x, t, net_params, A):
    x = np.asarray(x)
    A = np.asarray(A)
    net_params = np.asarray(net_params)

    if "nc" not in _CACHED:
        _CACHED["nc"] = _build_program()
    nc = _CACHED["nc"]

    in_maps = _prep_inputs(x, net_params, A)
    _CACHED["in_maps"] = in_maps
    res = run_bass_kernel_spmd(nc, in_maps, list(range(NCORES)))
    full = np.concatenate([res.results[c]["out"] for c in range(NCORES)],
                          axis=0).astype(np.float32)
    return np.ascontiguousarray(full.reshape(N, B, F).transpose(1, 0, 2))
